# revision 12
# baseline (speedup 1.0000x reference)
"""Trainium2 Bass kernel for nn_Net_12481174962490.

Data-parallel over batch (B=8 -> 8 NeuronCores, 1 image per core).
Training-mode BatchNorm batch statistics are exchanged with tiny AllReduces.

Key structural choices (vs naive translation of the reference):
  * conv1(256^2) + maxpool are computed in 2x2 phase space on a 128^2 grid,
    with all 9 taps folded into the contraction dim (K=36) host-side.
  * The 81 `_change_blur` channels are spatially constant per image, so they
    are never materialized: their contribution to res1_conv1 is a per-channel
    constant S[o] (added as eviction bias) plus analytic one-pixel border
    corrections (applied to PSUM before eviction), and to conv1x1_1 a bias.
  * conv2 folds its 5 horizontal taps into K=80 via 5 shifted stacked copies.
  * pixel_shuffle + up_conv3/up_conv4 stay on the 128^2 pre-shuffle grid
    (phase-decomposed weights); the final interleave happens on host.
  * All matmuls in bf16 (fp32 PSUM accumulation); BN stats in fp32 taken from
    PSUM tiles via bn_stats/bn_aggr.
"""
import os
import numpy as np
import ml_dtypes

os.environ.setdefault("JAX_PLATFORMS", "axon")
import jax

try:
    jax.config.update("jax_compilation_cache_dir", "/tmp/jax_bass_cache")
    jax.config.update("jax_persistent_cache_min_compile_time_secs", 0.0)
except Exception:
    pass

import concourse.bass as bass
import concourse.tile as tile
from concourse import bacc, mybir
from concourse import bass2jax

F32 = mybir.dt.float32
BF16 = mybir.dt.bfloat16
AF = mybir.ActivationFunctionType
ALU = mybir.AluOpType
BF = ml_dtypes.bfloat16

N_CORES = 8
EPS = 1e-5


# --------------------------------------------------------------------------
# host-side weight preparation
# --------------------------------------------------------------------------

def _phase_map(a, dy):
    """(output phase a, kernel tap dy) -> (src phase r, block shift du)."""
    v = a + dy - 1
    return v & 1, (v >> 1) if v >= 0 else -((-v + 1) >> 1)


def _prep_weights(p):
    """Transform reference params into device layouts. Returns dict[str, np]."""
    g = {}

    def w(name):
        return np.asarray(p[name], np.float32)

    # conv1 in phase space: K=36 (tap(du,dv) x input phase(r,s)),
    # M=128 (32*outphase + o), taps folded into K.
    w1 = w("conv1_w")  # [16,1,3,3]
    W1 = np.zeros((36, 128), np.float32)
    for a in range(2):
        for b in range(2):
            pph = a * 2 + b
            for dy in range(3):
                r, du = _phase_map(a, dy)
                for dx in range(3):
                    s, dv = _phase_map(b, dx)
                    t = (du + 1) * 3 + (dv + 1)
                    k = t * 4 + r * 2 + s
                    W1[k, 32 * pph:32 * pph + 16] += w1[:, 0, dy, dx]
    g["w1"] = W1.astype(BF)
    g["b1"] = w("conv1_b").reshape(16, 1)

    # conv2: K=80 (kx*16 + c), per-ky lhsT [80, 100]
    w2 = w("conv2_w")  # [100,16,5,5]
    W2 = np.zeros((80, 5, 100), np.float32)
    for kx in range(5):
        for ky in range(5):
            W2[kx * 16:(kx + 1) * 16, ky, :] = w2[:, :, ky, kx].T
    g["w2"] = W2.reshape(80, 5 * 100).astype(BF)
    g["b2"] = w("conv2_b").reshape(100, 1)

    def tap_layout(wt):  # [O,I,k,k] -> [I, k*k, O]
        O, I, kh, kw = wt.shape
        return np.ascontiguousarray(wt.transpose(1, 2, 3, 0).reshape(I, kh * kw, O)
                                    .transpose(0, 1, 2)).reshape(I, kh * kw * O)

    def tap_layout3(wt):  # [O,I,3,3] -> [I, 9, O] flat
        O, I, _, _ = wt.shape
        arr = np.zeros((I, 9, O), np.float32)
        for dy in range(3):
            for dx in range(3):
                arr[:, dy * 3 + dx, :] = wt[:, :, dy, dx].T
        return arr.reshape(I, 9 * O)

    g["w3"] = tap_layout3(w("conv3_w")).astype(BF)          # [100, 9*32]
    g["b3"] = w("conv3_b").reshape(32, 1)
    g["w4"] = tap_layout3(w("conv4_w")).astype(BF)          # [32, 9*9]
    g["b4"] = w("conv4_b").reshape(9, 1)
    g["w5"] = w("conv5_w").reshape(1, 9).T.astype(BF)       # [9, 1]
    g["b5"] = w("conv5_b").reshape(1, 1)

    w6 = w("res1_conv1_w")  # [256, 181, 3, 3]; input 0:81 blur, 81:181 real
    w6r = w6[:, 81:, :, :]
    arr = np.zeros((100, 9, 2, 128), np.float32)
    for dy in range(3):
        for dx in range(3):
            t = dy * 3 + dx
            for j in range(2):
                arr[:, t, j, :] = w6r[j * 128:(j + 1) * 128, :, dy, dx].T
    g["w6"] = arr.reshape(100, 9 * 2 * 128).astype(BF)
    g["b6"] = np.stack([w("res1_conv1_b")[:128], w("res1_conv1_b")[128:]], 1)  # [128,2]
    w6b = w6[:, :81, :, :]
    arrb = np.zeros((81, 9, 2, 128), np.float32)
    for dy in range(3):
        for dx in range(3):
            t = dy * 3 + dx
            for j in range(2):
                arrb[:, t, j, :] = w6b[j * 128:(j + 1) * 128, :, dy, dx].T
    g["w6b"] = arrb.reshape(81, 9 * 2 * 128).astype(BF)

    w7 = w("conv1x1_1_w")[:, :, 0, 0]  # [32, 181]
    g["w7"] = np.ascontiguousarray(w7[:, 81:].T).astype(BF)   # [100, 32]
    g["w7b"] = np.ascontiguousarray(w7[:, :81].T).astype(BF)  # [81, 32]
    g["b7"] = w("conv1x1_1_b").reshape(32, 1)

    w8 = w("res1_conv2_w")  # [32, 256, 3, 3]
    g["w8a"] = tap_layout3(w8[:, :128]).astype(BF)   # [128, 9*32]
    g["w8b"] = tap_layout3(w8[:, 128:]).astype(BF)
    g["b8"] = w("res1_conv2_b").reshape(32, 1)

    g["w9"] = tap_layout3(w("res2_conv1_w")).astype(BF)   # [32, 9*128]
    g["b9"] = w("res2_conv1_b").reshape(128, 1)
    g["w10"] = tap_layout3(w("res2_conv2_w")).astype(BF)  # [128, 9*64]
    g["b10"] = w("res2_conv2_b").reshape(64, 1)
    g["w11"] = np.ascontiguousarray(w("conv1x1_2_w")[:, :, 0, 0].T).astype(BF)  # [32,64]
    g["b11"] = w("conv1x1_2_b").reshape(64, 1)
    g["w12"] = tap_layout3(w("up_conv1_w")).astype(BF)    # [64, 9*30]
    g["b12"] = w("up_conv1_b").reshape(30, 1)
    g["w13"] = np.ascontiguousarray(w("up_conv2_w")[:, :, 0, 0].T).astype(BF)  # [30,16]
    g["b13"] = w("up_conv2_b").reshape(16, 1)

    # up_conv3 in pre-shuffle space: K=16 (c'*4 + 2*r1 + r2),
    # M=32 (outphase*8 + o), union taps (du,dv) in {-1,0,1}^2
    w14r = w("up_conv3_w")  # [8, 4, 3, 3]
    W14 = np.zeros((16, 9, 32), np.float32)
    for a in range(2):
        for b in range(2):
            pph = a * 2 + b
            for dy in range(3):
                r1, du = _phase_map(a, dy)
                for dx in range(3):
                    r2, dv = _phase_map(b, dx)
                    t = (du + 1) * 3 + (dv + 1)
                    for cp in range(4):
                        W14[cp * 4 + 2 * r1 + r2, t, pph * 8:pph * 8 + 8] += \
                            w14r[:, cp, dy, dx]
    g["w14"] = W14.reshape(16, 9 * 32).astype(BF)
    g["b14"] = np.tile(w("up_conv3_b"), 4).reshape(32, 1)

    w15r = w("up_conv4_w")[0, :, 0, 0]  # [8]
    W15 = np.zeros((32, 4), np.float32)
    for pph in range(4):
        W15[pph * 8:(pph + 1) * 8, pph] = w15r
    g["w15"] = W15.astype(BF)
    g["b15"] = np.full((4, 1), float(np.asarray(p["up_conv4_b"])[0]), np.float32)

    # BN affine params
    g["bn1_gb"] = np.stack([w("bn1_g"), w("bn1_b")], 1)            # [16,2]
    r1g, r1b = w("res1_bn1_g"), w("res1_bn1_b")
    g["r1b1_g"] = np.stack([r1g[:128], r1g[128:]], 1)              # [128,2]
    g["r1b1_b"] = np.stack([r1b[:128], r1b[128:]], 1)
    g["r1b2_gb"] = np.stack([w("res1_bn2_g"), w("res1_bn2_b")], 1)  # [32,2]
    g["r2b1_gb"] = np.stack([w("res2_bn1_g"), w("res2_bn1_b")], 1)  # [128,2]
    g["r2b2_gb"] = np.stack([w("res2_bn2_g"), w("res2_bn2_b")], 1)  # [64,2]
    for k, v in g.items():
        if v.dtype == np.float32:
            g[k] = np.ascontiguousarray(v)
    return g


def _prep_x(x_core):
    """x_core [1,256,256] f32 -> xp9 [36, 128*128] bf16 (taps x phases)."""
    x = np.asarray(x_core, np.float32)[0]
    out = np.zeros((9, 4, 130, 130), np.float32)
    # phase images with 1-px zero pad
    ph = np.zeros((4, 130, 130), np.float32)
    for r in range(2):
        for s in range(2):
            ph[r * 2 + s, 1:129, 1:129] = x[r::2, s::2]
    for du in (-1, 0, 1):
        for dv in (-1, 0, 1):
            t = (du + 1) * 3 + (dv + 1)
            # xp9[t][., y, x] = phase[., y+du, x+dv] on the 128 interior
            out[t, :, 1:129, 1:129] = ph[:, 1 + du:129 + du, 1 + dv:129 + dv]
    return np.ascontiguousarray(out[:, :, 1:129, 1:129].reshape(36, 128 * 128)).astype(BF)


# --------------------------------------------------------------------------
# device kernel build
# --------------------------------------------------------------------------

INPUT_SPECS = [
    ("xp9", [36, 128 * 128], BF16),
    ("w1", [36, 128], BF16), ("b1", [16, 1], F32),
    ("w2", [80, 5 * 100], BF16), ("b2", [100, 1], F32),
    ("w3", [100, 9 * 32], BF16), ("b3", [32, 1], F32),
    ("w4", [32, 9 * 9], BF16), ("b4", [9, 1], F32),
    ("w5", [9, 1], BF16), ("b5", [1, 1], F32),
    ("w6", [100, 9 * 2 * 128], BF16), ("b6", [128, 2], F32),
    ("w6b", [81, 9 * 2 * 128], BF16),
    ("w7", [100, 32], BF16), ("w7b", [81, 32], BF16), ("b7", [32, 1], F32),
    ("w8a", [128, 9 * 32], BF16), ("w8b", [128, 9 * 32], BF16), ("b8", [32, 1], F32),
    ("w9", [32, 9 * 128], BF16), ("b9", [128, 1], F32),
    ("w10", [128, 9 * 64], BF16), ("b10", [64, 1], F32),
    ("w11", [32, 64], BF16), ("b11", [64, 1], F32),
    ("w12", [64, 9 * 30], BF16), ("b12", [30, 1], F32),
    ("w13", [30, 16], BF16), ("b13", [16, 1], F32),
    ("w14", [16, 9 * 32], BF16), ("b14", [32, 1], F32),
    ("w15", [32, 4], BF16), ("b15", [4, 1], F32),
    ("bn1_gb", [16, 2], F32),
    ("r1b1_g", [128, 2], F32), ("r1b1_b", [128, 2], F32),
    ("r1b2_gb", [32, 2], F32), ("r2b1_gb", [128, 2], F32),
    ("r2b2_gb", [64, 2], F32),
]

DEBUG_TAPS = False  # extra outputs for stage-by-stage debugging


def build_nc():
    nc = bacc.Bacc("TRN2", target_bir_lowering=False, debug=False,
                   enable_asserts=True, num_devices=N_CORES)
    I = {}
    for name, shape, dt in INPUT_SPECS:
        I[name] = nc.dram_tensor(name, shape, dt, kind="ExternalInput").ap()
    out_sig = nc.dram_tensor("out_sig", [4, 128 * 128], F32,
                             kind="ExternalOutput").ap()
    dbg = {}
    if DEBUG_TAPS:
        for name, shape, ddt in [
            ("d_pooled", [16, 128 * 128], BF16),
            ("d_pooled2", [100, 66 * 66], BF16), ("d_blur", [81, 1], BF16),
            ("d_r1c1a", [128, 130 * 130], BF16), ("d_r1c1b", [128, 130 * 130], BF16),
            ("d_y3", [32, 128 * 128], BF16), ("d_z1", [32, 130 * 130], BF16),
            ("d_y4", [128, 130 * 130], BF16), ("d_z2", [64, 130 * 130], BF16),
            ("d_u2", [16, 130 * 130], BF16), ("d_u3", [32, 128 * 128], BF16),
        ]:
            dbg[name] = nc.dram_tensor(name, shape, ddt, kind="ExternalOutput").ap()

    with tile.TileContext(nc) as tc:
        _emit(nc, tc, I, out_sig, dbg)
    nc.compile()
    return nc


def _bn_post(nc, pool, arsum, gb_g, gb_b, name):
    """Given AllReduced (sum mean_i, sum ex2_i) [C,2] and affine g,b [C,1],
    return (s, t) scale/shift [C,1] f32 tiles: bn(x) = s*x + t."""
    C = arsum.shape[0]
    mu = pool.tile([C, 1], F32, name=f"mu_{name}")
    var = pool.tile([C, 1], F32, name=f"var_{name}")
    s = pool.tile([C, 1], F32, name=f"s_{name}")
    t = pool.tile([C, 1], F32, name=f"t_{name}")
    nc.vector.tensor_scalar_mul(mu[:], arsum[:, 0:1], 1.0 / N_CORES)
    nc.scalar.square(var[:], mu[:])
    nc.vector.tensor_scalar(var[:], var[:], -1.0, EPS, ALU.mult, ALU.add)  # eps - mu^2
    nc.vector.tensor_scalar(s[:], arsum[:, 1:2], 1.0 / N_CORES, 0.0, ALU.mult)
    nc.vector.tensor_tensor(var[:], var[:], s[:], ALU.add)  # var + eps
    nc.scalar.sqrt(s[:], var[:])
    nc.vector.reciprocal(s[:], s[:])                        # rstd
    nc.vector.tensor_tensor(s[:], s[:], gb_g, ALU.mult)     # s = g * rstd
    nc.vector.tensor_tensor(t[:], mu[:], s[:], ALU.mult)    # mu * s
    nc.vector.tensor_tensor(t[:], gb_b, t[:], ALU.subtract)  # t = b - mu*s
    return s, t


def _bn_payload(nc, pool, stats6, bias_ap, name):
    """bn_aggr over stats6 [C, T, 6]; returns payload [C,2] = (mean+bias, ex2)."""
    C = stats6.shape[0]
    mv = pool.tile([C, 2], F32, name=f"mv_{name}")
    pay = pool.tile([C, 2], F32, name=f"pay_{name}")
    tmp = pool.tile([C, 1], F32, name=f"tmp_{name}")
    nc.vector.bn_aggr(mv[:], stats6[:])
    if bias_ap is not None:
        nc.vector.tensor_tensor(pay[:, 0:1], mv[:, 0:1], bias_ap, ALU.add)
    else:
        nc.vector.tensor_copy(pay[:, 0:1], mv[:, 0:1])
    nc.scalar.square(tmp[:], pay[:, 0:1])
    nc.vector.tensor_tensor(pay[:, 1:2], mv[:, 1:2], tmp[:], ALU.add)
    return pay


def _allreduce(nc, dram, pay, name):
    C, Wd = pay.shape
    d_in = dram.tile([C, Wd], F32, name=f"ar_in_{name}")
    d_out = dram.tile([C, Wd], F32, name=f"ar_out_{name}")
    nc.sync.dma_start(d_in[:], pay[:])
    nc.gpsimd.collective_compute(
        "AllReduce", ALU.add, replica_groups=[list(range(N_CORES))],
        ins=[d_in.opt()], outs=[d_out.opt()])
    return d_out


def _zero_borders(nc, buf, H, W):
    """buf [C, H, W]: zero rows 0,H-1 and cols 0,W-1."""
    nc.vector.memset(buf[:, 0, :], 0.0)
    nc.vector.memset(buf[:, H - 1, :], 0.0)
    nc.vector.memset(buf[:, :, 0:1], 0.0)
    nc.vector.memset(buf[:, :, W - 1:W], 0.0)


def _emit(nc, tc, I, out_sig, dbg):
    import contextlib
    ctx = contextlib.ExitStack()
    wp = ctx.enter_context(tc.tile_pool(name="wp", bufs=1))
    sc = ctx.enter_context(tc.tile_pool(name="sc", bufs=1))      # scalars/stats
    pad = ctx.enter_context(tc.tile_pool(name="pad", bufs=2))    # [*,130,130]-ish
    flat = ctx.enter_context(tc.tile_pool(name="flat", bufs=2))  # [*,128,128]
    small = ctx.enter_context(tc.tile_pool(name="small", bufs=2))
    stg = ctx.enter_context(tc.tile_pool(name="stg", bufs=3))    # small staging
    stg2 = ctx.enter_context(tc.tile_pool(name="stg2", bufs=2))  # bigger staging
    psum = ctx.enter_context(tc.tile_pool(name="psum", bufs=4, space="PSUM"))
    pst = ctx.enter_context(tc.tile_pool(name="pst", bufs=2, space="PSUM"))
    dram = ctx.enter_context(tc.tile_pool(name="dram", bufs=1, space="DRAM"))

    # ---- load weights ----
    W = {}
    for name, shape, dt in INPUT_SPECS:
        if name == "xp9":
            continue
        t_ = wp.tile(shape, dt, name=f"w_{name}")
        nc.sync.dma_start(t_[:], I[name])
        W[name] = t_
    w2v = W["w2"][:].rearrange("k (t o) -> k t o", t=5)
    w3v = W["w3"][:].rearrange("k (t o) -> k t o", t=9)
    w4v = W["w4"][:].rearrange("k (t o) -> k t o", t=9)
    w6v = W["w6"][:].rearrange("k (t j o) -> k t j o", t=9, j=2)
    w6bv = W["w6b"][:].rearrange("k (t j o) -> k t j o", t=9, j=2)
    w8av = W["w8a"][:].rearrange("k (t o) -> k t o", t=9)
    w8bv = W["w8b"][:].rearrange("k (t o) -> k t o", t=9)
    w9v = W["w9"][:].rearrange("k (t o) -> k t o", t=9)
    w10v = W["w10"][:].rearrange("k (t o) -> k t o", t=9)
    w12v = W["w12"][:].rearrange("k (t o) -> k t o", t=9)
    w14v = W["w14"][:].rearrange("k (t o) -> k t o", t=9)

    # ==================== Stage A: conv1 + pool + bn1 stats ====================
    pooled = flat.tile([16, 128, 128], BF16, tag="flat", name="pooled")
    statsA = sc.tile([16, 32, 6], F32, name="statsA")
    xp9v = I["xp9"].rearrange("k (h w) -> k h w", h=128)
    for t in range(32):
        rhs1 = stg.tile([36, 4, 128], BF16, tag="rhsS", name="rhs1")
        nc.sync.dma_start(rhs1[:], xp9v[:, 4 * t:4 * t + 4, :])
        ps = psum.tile([128, 512], F32, tag="mm", name="psA")
        nc.tensor.matmul(ps[:], W["w1"][:], rhs1[:], start=True, stop=True)
        # conv1 bias is NOT added here: bias commutes with the phase max and is
        # folded into the bn1 apply below (and into the bn1 mean payload).
        stB = stg2.tile([16, 2, 512], BF16, tag="stB", name="stB")
        nc.scalar.copy(stB[:, 0], ps[0:16, :])
        nc.scalar.copy(stB[:, 1], ps[64:80, :])
        nc.vector.tensor_tensor(stB[:, 0], stB[:, 0], ps[32:48, :], ALU.max)
        nc.vector.tensor_tensor(stB[:, 1], stB[:, 1], ps[96:112, :], ALU.max)
        pr = pooled[:, 4 * t:4 * t + 4, :]
        nc.vector.tensor_tensor(pr, stB[:, 0], stB[:, 1], ALU.max)
        nc.vector.bn_stats(statsA[:, t], pr.rearrange("c h w -> c (h w)"))

    payA = _bn_payload(nc, sc, statsA, W["b1"][:, 0:1], "A")
    arA_out = _allreduce(nc, dram, payA, "A")
    arA = sc.tile([16, 2], F32, name="arA")
    nc.sync.dma_start(arA[:], arA_out[:])
    s1, t1 = _bn_post(nc, sc, arA, W["bn1_gb"][:, 0:1], W["bn1_gb"][:, 1:2], "A")
    # fold conv1 bias into shift: relu(s*(x+b1) + t) = relu(s*x + (t + s*b1))
    tb = sc.tile([16, 1], F32, name="tbA")
    nc.vector.tensor_tensor(tb[:], s1[:], W["b1"][:, 0:1], ALU.mult)
    nc.vector.tensor_tensor(t1[:], t1[:], tb[:], ALU.add)
    # apply bn+relu in place on pooled
    nc.scalar.activation(pooled[:], pooled[:], AF.Relu,
                         bias=t1[:, 0:1], scale=s1[:, 0:1])
    if DEBUG_TAPS:
        nc.sync.dma_start(dbg["d_pooled"].rearrange("c (h w) -> c h w", h=128),
                          pooled[:])

    # ==================== Stage B: conv2 (+ keep downsample in DRAM) =========
    xs2 = pad.tile([80, 132, 128], BF16, tag="pad", name="xs2")
    nc.vector.memset(xs2[:], 0.0)
    for kx in range(5):
        x0 = max(0, 2 - kx)
        x1 = min(128, 130 - kx)
        nc.sync.dma_start(xs2[kx * 16:(kx + 1) * 16, 2:130, x0:x1],
                          pooled[:, :, x0 + kx - 2:x1 + kx - 2])

    c2pad = dram.tile([100, 130, 130], BF16, name="c2pad")
    zrow = sc.tile([100, 130], BF16, name="zrow")
    nc.vector.memset(zrow[:], 0.0)
    nc.sync.dma_start(c2pad[:, 0, :], zrow[:])
    nc.sync.dma_start(c2pad[:, 129, :], zrow[:])
    hmax = flat.tile([100, 128, 64], BF16, tag="flat", name="hmax")
    for t in range(32):
        ps = psum.tile([100, 512], F32, tag="mm", name="psB")
        for ky in range(5):
            nc.tensor.matmul(ps[:], w2v[:, ky], xs2[:, 4 * t + ky:4 * t + ky + 4, :],
                             start=(ky == 0), stop=(ky == 4))
        c2s = stg2.tile([100, 4, 130], BF16, tag="evs", name="c2s")
        nc.vector.memset(c2s[:, :, 0:1], 0.0)
        nc.vector.memset(c2s[:, :, 129:130], 0.0)
        nc.scalar.activation(c2s[:, :, 1:129], ps[:], AF.Identity,
                             bias=W["b2"][:, 0:1], scale=1.0)
        nc.sync.dma_start(c2pad[:, 1 + 4 * t:1 + 4 * t + 4, :], c2s[:])
        nc.vector.tensor_tensor(hmax[:, 4 * t:4 * t + 4, :],
                                c2s[:, :, 1:129:2], c2s[:, :, 2:130:2], ALU.max)

    # pool2 + relu -> pooled2 [100, 66, 66]
    pooled2 = small.tile([100, 66, 66], BF16, tag="sm", name="pooled2")
    _zero_borders(nc, pooled2, 66, 66)
    p2i = pooled2[:, 1:65, 1:65]
    nc.vector.tensor_tensor(p2i, hmax[:, 0::2, :], hmax[:, 1::2, :], ALU.max)
    nc.vector.tensor_scalar_max(p2i, p2i, 0.0)
    if DEBUG_TAPS:
        nc.sync.dma_start(dbg["d_pooled2"].rearrange("c (h w) -> c h w", h=66),
                          pooled2[:])

    # ==================== Stage C: conv3/4/5, blur, T machinery ==============
    c3out = small.tile([32, 66, 66], BF16, tag="sm", name="c3out")
    _zero_borders(nc, c3out, 66, 66)
    for t in range(8):
        ps = psum.tile([32, 512], F32, tag="mm", name="psC3")
        for dy in range(3):
            for dx in range(3):
                nc.tensor.matmul(ps[:], w3v[:, dy * 3 + dx],
                                 pooled2[:, 8 * t + dy:8 * t + dy + 8, dx:dx + 64],
                                 start=(dy == 0 and dx == 0),
                                 stop=(dy == 2 and dx == 2))
        nc.scalar.activation(c3out[:, 1 + 8 * t:1 + 8 * t + 8, 1:65], ps[:],
                             AF.Identity, bias=W["b3"][:, 0:1], scale=1.0)

    c4out = small.tile([9, 66, 66], BF16, tag="sm", name="c4out")
    _zero_borders(nc, c4out, 66, 66)
    for t in range(8):
        ps = psum.tile([32, 512], F32, tag="mm", name="psC4")
        for dy in range(3):
            for dx in range(3):
                nc.tensor.matmul(ps[:9], w4v[:, dy * 3 + dx],
                                 c3out[:, 8 * t + dy:8 * t + dy + 8, dx:dx + 64],
                                 start=(dy == 0 and dx == 0),
                                 stop=(dy == 2 and dx == 2))
        nc.scalar.activation(c4out[:, 1 + 8 * t:1 + 8 * t + 8, 1:65], ps[:9],
                             AF.Identity, bias=W["b4"][:, 0:1], scale=1.0)

    blurb = sc.tile([81, 1], BF16, name="blurb")
    for t in range(8):
        ps = psum.tile([32, 512], F32, tag="mm", name="psC5")
        nc.tensor.matmul(ps[:1], W["w5"][:], c4out[:, 1 + 8 * t:1 + 8 * t + 8, 1:65],
                         start=True, stop=True)
        c5st = stg2.tile([1, 8, 64], BF16, tag="evs", name="c5st")
        nc.scalar.activation(c5st[:], ps[:1], AF.Relu,
                             bias=W["b5"][:, 0:1], scale=1.0)
        # resize rows needed from this tile: 7a in [8t, 8t+8)
        for a in range(9):
            if 7 * a // 8 == t:
                src = bass.AP(tensor=c5st[:].tensor, offset=c5st[:].offset,
                              ap=[[c5st[:].ap[0][0], 1],
                                  [64, 1], [7, 9]])
                src = bass.AP(tensor=src.tensor,
                              offset=src.offset + (7 * a - 8 * t) * 64,
                              ap=src.ap)
                nc.sync.dma_start(blurb[a * 9:(a + 1) * 9, 0:1], src)
    if DEBUG_TAPS:
        nc.sync.dma_start(dbg["d_blur"], blurb[:])

    # T_j[o, tap] = sum_k blur[k] * w6b[k, tap, j, o]
    Ts, biasD = [], []
    for j in range(2):
        psT = pst.tile([128, 16], F32, tag="tiny", name=f"psT{j}")
        for t9 in range(9):
            nc.tensor.matmul(psT[:, t9:t9 + 1], w6bv[:, t9, j], blurb[:],
                             start=True, stop=True, skip_group_check=True)
        Tj = sc.tile([128, 9], F32, name=f"T{j}")
        nc.scalar.copy(Tj[:], psT[:, 0:9])
        Ts.append(Tj)
        Sj = sc.tile([128, 1], F32, name=f"S{j}")
        nc.vector.tensor_reduce(Sj[:], Tj[:], mybir.AxisListType.X, ALU.add)
        bD = sc.tile([128, 1], F32, name=f"biasD{j}")
        nc.vector.tensor_tensor(bD[:], W["b6"][:, j:j + 1], Sj[:], ALU.add)
        biasD.append(bD)
    edgeT = []  # per j: (rowT, rowB, colL, colR)
    for j in range(2):
        Tj = Ts[j]
        rT = sc.tile([128, 1], F32, name=f"rT{j}")
        rB = sc.tile([128, 1], F32, name=f"rB{j}")
        cL = sc.tile([128, 1], F32, name=f"cL{j}")
        cR = sc.tile([128, 1], F32, name=f"cR{j}")
        nc.vector.tensor_reduce(rT[:], Tj[:, 0:3], mybir.AxisListType.X, ALU.add)
        nc.vector.tensor_reduce(rB[:], Tj[:, 6:9], mybir.AxisListType.X, ALU.add)
        nc.vector.tensor_reduce(cL[:], Tj[:, 0:9:3], mybir.AxisListType.X, ALU.add)
        nc.vector.tensor_reduce(cR[:], Tj[:, 2:9:3], mybir.AxisListType.X, ALU.add)
        edgeT.append((rT, rB, cL, cR))

    # conv1x1_1 blur constant -> bias
    psQ = pst.tile([128, 16], F32, tag="tiny", name="psQ")
    nc.tensor.matmul(psQ[:32, 0:1], W["w7b"][:], blurb[:], start=True, stop=True)
    bias7 = sc.tile([32, 1], F32, name="bias7")
    nc.scalar.copy(bias7[:], psQ[:32, 0:1])
    nc.vector.tensor_tensor(bias7[:], bias7[:], W["b7"][:, 0:1], ALU.add)

    # ==================== Stage D: res1_conv1 ================================
    r1c1 = [pad.tile([128, 130, 130], BF16, tag="pad", name=f"r1c1_{j}")
            for j in range(2)]
    statsD = [sc.tile([128, 32, 6], F32, name=f"statsD{j}") for j in range(2)]
    for j in range(2):
        _zero_borders(nc, r1c1[j], 130, 130)
    for t in range(32):
        rhs6 = stg2.tile([100, 6, 130], BF16, tag="rhs6", name="rhs6")
        nc.sync.dma_start(rhs6[:], c2pad[:, 4 * t:4 * t + 6, :])
        for j in range(2):
            ps = psum.tile([128, 512], F32, tag="mm", name="psD")
            for dy in range(3):
                for dx in range(3):
                    nc.tensor.matmul(ps[:], w6v[:, dy * 3 + dx, j],
                                     rhs6[:, dy:dy + 4, dx:dx + 128],
                                     start=(dy == 0 and dx == 0),
                                     stop=(dy == 2 and dx == 2))
            # blur border corrections on PSUM (before stats + eviction)
            rT, rB, cL, cR = edgeT[j]
            psv = ps[:].rearrange("c (h w) -> c h w", h=4)
            nc.vector.tensor_scalar(psv[:, :, 0:1], psv[:, :, 0:1], cL[:, 0:1],
                                    None, ALU.subtract)
            nc.vector.tensor_scalar(psv[:, :, 127:128], psv[:, :, 127:128],
                                    cR[:, 0:1], None, ALU.subtract)
            if t == 0:
                nc.vector.tensor_scalar(psv[:, 0, :], psv[:, 0, :], rT[:, 0:1],
                                        None, ALU.subtract)
                nc.vector.tensor_scalar(psv[:, 0, 0:1], psv[:, 0, 0:1],
                                        Ts[j][:, 0:1], None, ALU.add)
                nc.vector.tensor_scalar(psv[:, 0, 127:128], psv[:, 0, 127:128],
                                        Ts[j][:, 2:3], None, ALU.add)
            if t == 31:
                nc.vector.tensor_scalar(psv[:, 3, :], psv[:, 3, :], rB[:, 0:1],
                                        None, ALU.subtract)
                nc.vector.tensor_scalar(psv[:, 3, 0:1], psv[:, 3, 0:1],
                                        Ts[j][:, 6:7], None, ALU.add)
                nc.vector.tensor_scalar(psv[:, 3, 127:128], psv[:, 3, 127:128],
                                        Ts[j][:, 8:9], None, ALU.add)
            nc.vector.bn_stats(statsD[j][:, t], ps[:])
            nc.scalar.activation(r1c1[j][:, 1 + 4 * t:1 + 4 * t + 4, 1:129], ps[:],
                                 AF.Identity, bias=biasD[j][:, 0:1], scale=1.0)

    payD0 = _bn_payload(nc, sc, statsD[0], biasD[0][:, 0:1], "D0")
    payD1 = _bn_payload(nc, sc, statsD[1], biasD[1][:, 0:1], "D1")
    payD = sc.tile([128, 4], F32, name="payD")
    nc.vector.tensor_copy(payD[:, 0:2], payD0[:])
    nc.vector.tensor_copy(payD[:, 2:4], payD1[:])
    arD_out = _allreduce(nc, dram, payD, "D")
    arD = sc.tile([128, 4], F32, name="arD")
    nc.sync.dma_start(arD[:], arD_out[:])
    for j in range(2):
        s2, t2 = _bn_post(nc, sc, arD[:, 2 * j:2 * j + 2],
                          W["r1b1_g"][:, j:j + 1], W["r1b1_b"][:, j:j + 1],
                          f"D{j}")
        nc.scalar.activation(r1c1[j][:, 1:129, 1:129], r1c1[j][:, 1:129, 1:129],
                             AF.Relu, bias=t2[:, 0:1], scale=s2[:, 0:1])
    if DEBUG_TAPS:
        for j, nm in enumerate(["d_r1c1a", "d_r1c1b"]):
            nc.sync.dma_start(dbg[nm].rearrange("c (h w) -> c h w", h=130),
                              r1c1[j][:])

    # ==================== Stage E: res1_conv2 + conv1x1_1 + z1 ===============
    y3 = flat.tile([32, 128, 128], BF16, tag="flat", name="y3")
    statsE = sc.tile([32, 32, 6], F32, name="statsE")
    for t in range(32):
        ps = psum.tile([32, 512], F32, tag="mm", name="psE")
        first = True
        for j, wv in ((0, w8av), (1, w8bv)):
            for dy in range(3):
                for dx in range(3):
                    nc.tensor.matmul(ps[:], wv[:, dy * 3 + dx],
                                     r1c1[j][:, 4 * t + dy:4 * t + dy + 4, dx:dx + 128],
                                     start=first, stop=(j == 1 and dy == 2 and dx == 2))
                    first = False
        nc.vector.bn_stats(statsE[:, t], ps[:])
        nc.scalar.activation(y3[:, 4 * t:4 * t + 4, :], ps[:], AF.Identity,
                             bias=W["b8"][:, 0:1], scale=1.0)
    payE = _bn_payload(nc, sc, statsE, W["b8"][:, 0:1], "E")
    arE_out = _allreduce(nc, dram, payE, "E")

    q1 = flat.tile([32, 128, 128], BF16, tag="flat", name="q1")
    for t in range(32):
        rhs7 = stg.tile([100, 4, 128], BF16, tag="rhsS", name="rhs7")
        nc.sync.dma_start(rhs7[:], c2pad[:, 1 + 4 * t:1 + 4 * t + 4, 1:129])
        ps = psum.tile([32, 512], F32, tag="mm", name="psQ1")
        nc.tensor.matmul(ps[:], W["w7"][:], rhs7[:], start=True, stop=True)
        nc.scalar.activation(q1[:, 4 * t:4 * t + 4, :], ps[:], AF.Identity,
                             bias=bias7[:, 0:1], scale=1.0)

    arE = sc.tile([32, 2], F32, name="arE")
    nc.sync.dma_start(arE[:], arE_out[:])
    s3, t3 = _bn_post(nc, sc, arE, W["r1b2_gb"][:, 0:1], W["r1b2_gb"][:, 1:2], "E")
    z1 = pad.tile([32, 130, 130], BF16, tag="pad", name="z1")
    _zero_borders(nc, z1, 130, 130)
    z1i = z1[:, 1:129, 1:129]
    nc.scalar.activation(z1i, y3[:], AF.Identity, bias=t3[:, 0:1], scale=s3[:, 0:1])
    nc.vector.tensor_tensor(z1i, z1i, q1[:], ALU.add)
    if DEBUG_TAPS:
        nc.sync.dma_start(dbg["d_z1"].rearrange("c (h w) -> c h w", h=130), z1[:])
        nc.sync.dma_start(dbg["d_y3"].rearrange("c (h w) -> c h w", h=128), y3[:])

    # ==================== Stage F: res2_conv1 ================================
    y4 = pad.tile([128, 130, 130], BF16, tag="pad", name="y4")
    _zero_borders(nc, y4, 130, 130)
    statsF = sc.tile([128, 32, 6], F32, name="statsF")
    for t in range(32):
        ps = psum.tile([128, 512], F32, tag="mm", name="psF")
        for dy in range(3):
            for dx in range(3):
                nc.tensor.matmul(ps[:], w9v[:, dy * 3 + dx],
                                 z1[:, 4 * t + dy:4 * t + dy + 4, dx:dx + 128],
                                 start=(dy == 0 and dx == 0),
                                 stop=(dy == 2 and dx == 2))
        nc.vector.bn_stats(statsF[:, t], ps[:])
        nc.scalar.activation(y4[:, 1 + 4 * t:1 + 4 * t + 4, 1:129], ps[:],
                             AF.Identity, bias=W["b9"][:, 0:1], scale=1.0)
    payF = _bn_payload(nc, sc, statsF, W["b9"][:, 0:1], "F")
    arF_out = _allreduce(nc, dram, payF, "F")
    arF = sc.tile([128, 2], F32, name="arF")
    nc.sync.dma_start(arF[:], arF_out[:])
    s4, t4 = _bn_post(nc, sc, arF, W["r2b1_gb"][:, 0:1], W["r2b1_gb"][:, 1:2], "F")
    nc.scalar.activation(y4[:, 1:129, 1:129], y4[:, 1:129, 1:129], AF.Relu,
                         bias=t4[:, 0:1], scale=s4[:, 0:1])

    # ==================== Stage G: res2_conv2 + conv1x1_2 + z2 ===============
    y5 = flat.tile([64, 128, 128], BF16, tag="flat", name="y5")
    statsG = sc.tile([64, 32, 6], F32, name="statsG")
    for t in range(32):
        ps = psum.tile([64, 512], F32, tag="mm", name="psG")
        for dy in range(3):
            for dx in range(3):
                nc.tensor.matmul(ps[:], w10v[:, dy * 3 + dx],
                                 y4[:, 4 * t + dy:4 * t + dy + 4, dx:dx + 128],
                                 start=(dy == 0 and dx == 0),
                                 stop=(dy == 2 and dx == 2))
        nc.vector.bn_stats(statsG[:, t], ps[:])
        nc.scalar.activation(y5[:, 4 * t:4 * t + 4, :], ps[:], AF.Identity,
                             bias=W["b10"][:, 0:1], scale=1.0)
    payG = _bn_payload(nc, sc, statsG, W["b10"][:, 0:1], "G")
    arG_out = _allreduce(nc, dram, payG, "G")

    q2 = flat.tile([64, 128, 128], BF16, tag="flat", name="q2")
    for t in range(32):
        ps = psum.tile([64, 512], F32, tag="mm", name="psQ2")
        nc.tensor.matmul(ps[:], W["w11"][:], z1[:, 1 + 4 * t:1 + 4 * t + 4, 1:129],
                         start=True, stop=True)
        nc.scalar.activation(q2[:, 4 * t:4 * t + 4, :], ps[:], AF.Identity,
                             bias=W["b11"][:, 0:1], scale=1.0)

    arG = sc.tile([64, 2], F32, name="arG")
    nc.sync.dma_start(arG[:], arG_out[:])
    s5, t5 = _bn_post(nc, sc, arG, W["r2b2_gb"][:, 0:1], W["r2b2_gb"][:, 1:2], "G")
    z2 = pad.tile([64, 130, 130], BF16, tag="pad", name="z2")
    _zero_borders(nc, z2, 130, 130)
    z2i = z2[:, 1:129, 1:129]
    nc.scalar.activation(z2i, y5[:], AF.Identity, bias=t5[:, 0:1], scale=s5[:, 0:1])
    nc.vector.tensor_tensor(z2i, z2i, q2[:], ALU.add)
    if DEBUG_TAPS:
        nc.sync.dma_start(dbg["d_z2"].rearrange("c (h w) -> c h w", h=130), z2[:])
        nc.sync.dma_start(dbg["d_y4"].rearrange("c (h w) -> c h w", h=130), y4[:])

    # ==================== Stage H: up_conv1, up_conv2 ========================
    u1 = flat.tile([30, 128, 128], BF16, tag="flat", name="u1")
    for t in range(32):
        ps = psum.tile([32, 512], F32, tag="mm", name="psH")
        for dy in range(3):
            for dx in range(3):
                nc.tensor.matmul(ps[:30], w12v[:, dy * 3 + dx],
                                 z2[:, 4 * t + dy:4 * t + dy + 4, dx:dx + 128],
                                 start=(dy == 0 and dx == 0),
                                 stop=(dy == 2 and dx == 2))
        nc.scalar.activation(u1[:, 4 * t:4 * t + 4, :], ps[:30], AF.Relu,
                             bias=W["b12"][:, 0:1], scale=1.0)

    u2 = pad.tile([16, 130, 130], BF16, tag="pad", name="u2")
    _zero_borders(nc, u2, 130, 130)
    for t in range(32):
        ps = psum.tile([16, 512], F32, tag="mm", name="psU2")
        nc.tensor.matmul(ps[:], W["w13"][:], u1[:, 4 * t:4 * t + 4, :],
                         start=True, stop=True)
        nc.scalar.activation(u2[:, 1 + 4 * t:1 + 4 * t + 4, 1:129], ps[:],
                             AF.Identity, bias=W["b13"][:, 0:1], scale=1.0)
    if DEBUG_TAPS:
        nc.sync.dma_start(dbg["d_u2"].rearrange("c (h w) -> c h w", h=130), u2[:])

    # ==================== Stage I: up_conv3, up_conv4 + sigmoid ==============
    u3 = flat.tile([32, 128, 128], BF16, tag="flat", name="u3")
    for t in range(32):
        ps = psum.tile([32, 512], F32, tag="mm", name="psI")
        for du in range(3):
            for dv in range(3):
                nc.tensor.matmul(ps[:], w14v[:, du * 3 + dv],
                                 u2[:, 4 * t + du:4 * t + du + 4, dv:dv + 128],
                                 start=(du == 0 and dv == 0),
                                 stop=(du == 2 and dv == 2))
        nc.scalar.activation(u3[:, 4 * t:4 * t + 4, :], ps[:], AF.Relu,
                             bias=W["b14"][:, 0:1], scale=1.0)
    if DEBUG_TAPS:
        nc.sync.dma_start(dbg["d_u3"].rearrange("c (h w) -> c h w", h=128), u3[:])

    osv = out_sig.rearrange("p (h w) -> p h w", h=128)
    for t in range(32):
        ps = psum.tile([4, 512], F32, tag="mm", name="psO")
        nc.tensor.matmul(ps[:], W["w15"][:], u3[:, 4 * t:4 * t + 4, :],
                         start=True, stop=True)
        sg = stg2.tile([4, 4, 128], F32, tag="sg", name="sg")
        nc.scalar.activation(sg[:], ps[:], AF.Sigmoid, bias=W["b15"][:, 0:1],
                             scale=1.0)
        nc.sync.dma_start(osv[:, 4 * t:4 * t + 4, :], sg[:])

    ctx.close()


# --------------------------------------------------------------------------
# runner (cached jit across calls)
# --------------------------------------------------------------------------

_CACHE = {}


class _Runner:
    def __init__(self):
        from jax.experimental.shard_map import shard_map
        from jax.sharding import Mesh, PartitionSpec

        self.nc = build_nc()
        bass2jax.install_neuronx_cc_hook()
        nc = self.nc
        partition_name = (nc.partition_id_tensor.name
                          if nc.partition_id_tensor else None)
        in_names, out_names, out_avals, zero_outs = [], [], [], []
        for alloc in nc.m.functions[0].allocations:
            if not isinstance(alloc, mybir.MemoryLocationSet):
                continue
            name = alloc.memorylocations[0].name
            if alloc.kind == "ExternalInput":
                if name != partition_name:
                    in_names.append(name)
            elif alloc.kind == "ExternalOutput":
                shape = tuple(alloc.tensor_shape)
                dtype = mybir.dt.np(alloc.dtype)
                out_names.append(name)
                out_avals.append(jax.core.ShapedArray(shape, dtype))
                zero_outs.append(np.zeros(shape, dtype))
        self.in_names = list(in_names)
        self.out_names = out_names
        self.zero_outs = zero_outs
        n_params = len(in_names)
        all_names = in_names + out_names
        if partition_name is not None:
            all_names = all_names + [partition_name]

        def _body(*args):
            operands = list(args)
            if partition_name is not None:
                operands.append(bass2jax.partition_id_tensor())
            outs = bass2jax._bass_exec_p.bind(
                *operands,
                out_avals=tuple(out_avals),
                in_names=tuple(all_names),
                out_names=tuple(out_names),
                lowering_input_output_aliases=(),
                sim_require_finite=False,
                sim_require_nnan=False,
                nc=nc,
            )
            return tuple(outs)

        devices = jax.devices()[:N_CORES]
        assert len(devices) == N_CORES
        self.mesh = Mesh(np.asarray(devices), ("core",))
        in_specs = (PartitionSpec("core"),) * (n_params + len(out_names))
        out_specs = (PartitionSpec("core"),) * len(out_names)
        donate = tuple(range(n_params, n_params + len(out_names)))
        self.call = jax.jit(
            shard_map(_body, mesh=self.mesh, in_specs=in_specs,
                      out_specs=out_specs, check_rep=False),
            donate_argnums=donate, keep_unused=True)

    def run(self, in_maps):
        concat_in = [
            np.concatenate([np.asarray(in_maps[c][n]) for c in range(N_CORES)], 0)
            for n in self.in_names
        ]
        concat_zeros = [
            np.zeros((N_CORES * z.shape[0], *z.shape[1:]), z.dtype)
            for z in self.zero_outs
        ]
        outs = self.call(*concat_in, *concat_zeros)
        res = []
        for c in range(N_CORES):
            d = {}
            for i, n in enumerate(self.out_names):
                full = np.asarray(outs[i])
                d[n] = full.reshape(N_CORES, -1)[c].reshape(
                    self.zero_outs[i].shape)
            res.append(d)
        return res


def _get_runner():
    if "r" not in _CACHE:
        _CACHE["r"] = _Runner()
    return _CACHE["r"]


def make_in_maps(x, params):
    g = _prep_weights(params)
    in_maps = []
    for c in range(N_CORES):
        m = dict(g)
        m["xp9"] = _prep_x(np.asarray(x)[c])
        in_maps.append(m)
    return in_maps


def assemble(results):
    out = np.zeros((N_CORES, 1, 256, 256), np.float32)
    for c in range(N_CORES):
        ph = results[c]["out_sig"].reshape(4, 128, 128)
        for a in range(2):
            for b in range(2):
                out[c, 0, a::2, b::2] = ph[a * 2 + b]
    return out


def kernel(x, params):
    runner = _get_runner()
    in_maps = make_in_maps(x, params)
    return assemble(runner.run(in_maps))
